# revision 18
# baseline (speedup 1.0000x reference)
"""Trainium2 Bass kernel for nn_Block_1589137900259 (dense transformer block).

Sharding over 8 NeuronCores: 2 head-groups (6 heads each) x 4 batches.
Core c: hg = c // 4 (heads 6*hg .. 6*hg+5), bg = c % 4 (batch bg).

Per core: LN1 on its batch, per-head QKV projections, causal attention,
partial c_proj summed over local heads (+ residual x on hg==0 cores),
then a 2-rank ReduceScatter over {c, c+4 mod 8} pairs gives each core a
512-token slice; LN2 + MLP run token-parallel; host reassembles.

Layouts (SBUF partition dim first):
  xT / qT / kT feature-major [f, t] (bf16), v token-major [t, f] (bf16).
  scores computed [s, t] (softmax reductions on free axis), exp blocks
  transposed on PE with a diag(1/denom) right-operand (fuses the softmax
  normalization), giving expT [t, s] for the attn@v matmul which directly
  yields headsT [f, s]; c_proj consumes headsT and produces token-major
  mha. MLP: hT = gelu(Wfc^T @ y2T) feature-major, mlp token-major.
All matmul operands bf16 (fp32 PSUM accumulate); trunk (LN, residual,
softmax stats, reductions) fp32.
"""

import os
import numpy as np
import ml_dtypes


def _env(k, d):
    return int(os.environ.get(k, d))


def _abl(part):
    return part in os.environ.get("KB_ABLATE", "").split(",")

import concourse.bacc as bacc
import concourse.mybir as mybir
import concourse.tile as tile
from concourse.bass_utils import run_bass_kernel_spmd
from concourse.masks import make_identity, make_causal_mask

F32 = mybir.dt.float32
BF16 = mybir.dt.bfloat16
AF = mybir.ActivationFunctionType

N_CORES = 8
GROUPS = [[0, 4], [1, 5], [2, 6], [3, 7]]

N, S, E, H = 4, 1024, 768, 12
NH = 6          # heads per core
T = 1024        # tokens per core (one batch)
TT = T // 128   # 8 token tiles
ET = E // 128   # 6 feature tiles
FH = 4 * E      # 3072
FHT = FH // 128  # 24
EPS = 1e-5
SCALE = float(1.0 / np.sqrt(np.float32(E)))
MASK_VAL = -1.0e5
# output-feature chunks for 768-wide matmul outputs (one PSUM bank each)
EO_CHUNKS = [(0, 512), (512, 256)]

_NC = None


def _layer_norm_tile(nc, pool, xt, g_b, b_b, out_ap, eps_t, apply_gb):
    """LN over the free axis of xt [128, 768] f32 -> out_ap [128, 768] f32."""
    stats = pool.tile([128, 3, 6], F32, tag="ln_stats")
    for sg in range(3):
        nc.vector.bn_stats(out=stats[:, sg, :], in_=xt[:, 256 * sg : 256 * (sg + 1)])
    mv = pool.tile([128, 2], F32, tag="ln_mv")
    nc.vector.bn_aggr(out=mv[:], in_=stats[:])
    sd = pool.tile([128, 1], F32, tag="ln_sd")
    nc.scalar.activation(out=sd[:], in_=mv[:, 1:2], func=AF.Sqrt, bias=eps_t[:])
    rstd = pool.tile([128, 1], F32, tag="ln_rstd")
    nc.vector.reciprocal(out=rstd[:], in_=sd[:])
    nc.vector.tensor_scalar(out=out_ap, in0=xt[:], scalar1=mv[:, 0:1],
                            scalar2=rstd[:], op0=mybir.AluOpType.subtract,
                            op1=mybir.AluOpType.mult)
    if apply_gb:
        nc.vector.tensor_mul(out_ap, out_ap, g_b[:])
        nc.vector.tensor_add(out_ap, out_ap, b_b[:])


def _build(apply_gb=True):
    nc = bacc.Bacc("TRN2", target_bir_lowering=False, debug=False,
                   num_devices=N_CORES)

    x_in = nc.dram_tensor("x_in", [T, E], F32, kind="ExternalInput")
    g1 = nc.dram_tensor("g1", [E], F32, kind="ExternalInput")
    b1 = nc.dram_tensor("b1", [E], F32, kind="ExternalInput")
    g2 = nc.dram_tensor("g2", [E], F32, kind="ExternalInput")
    b2 = nc.dram_tensor("b2", [E], F32, kind="ExternalInput")
    wq = nc.dram_tensor("wq", [NH, E, E], BF16, kind="ExternalInput")
    wk = nc.dram_tensor("wk", [NH, E, E], BF16, kind="ExternalInput")
    wv = nc.dram_tensor("wv", [NH, E, E], BF16, kind="ExternalInput")
    wc = nc.dram_tensor("wc", [NH, E, E], BF16, kind="ExternalInput")
    bq = nc.dram_tensor("bq", [NH, E], F32, kind="ExternalInput")
    bk = nc.dram_tensor("bk", [NH, E], F32, kind="ExternalInput")
    bv = nc.dram_tensor("bv", [NH, E], F32, kind="ExternalInput")
    bc = nc.dram_tensor("bc", [E], F32, kind="ExternalInput")
    xw = nc.dram_tensor("xw", [1], F32, kind="ExternalInput")
    wfc = nc.dram_tensor("wfc", [E, FH], BF16, kind="ExternalInput")
    bfc = nc.dram_tensor("bfc", [FH], F32, kind="ExternalInput")
    wp = nc.dram_tensor("wp", [FH, E], BF16, kind="ExternalInput")
    bp = nc.dram_tensor("bp", [E], F32, kind="ExternalInput")
    out = nc.dram_tensor("out", [512, E], F32, kind="ExternalOutput")
    import os
    _dbg = os.environ.get("KBLOCK_DEBUG", "") == "1"
    if _dbg:
        dbg_contrib = nc.dram_tensor("dbg_contrib", [T, E], F32,
                                     kind="ExternalOutput")
        dbg_x = nc.dram_tensor("dbg_x", [T, E], F32, kind="ExternalOutput")
        dbg_qT = nc.dram_tensor("dbg_qT", [E, T], F32, kind="ExternalOutput")
        dbg_att = nc.dram_tensor("dbg_att", [128, T], F32, kind="ExternalOutput")

    def bcast(v_ap, n=128):
        import concourse.bass as bass
        return bass.AP(tensor=v_ap.tensor, offset=v_ap.offset,
                       ap=[[0, n]] + list(v_ap.ap))

    with tile.TileContext(nc) as tc:
        from contextlib import ExitStack
        with ExitStack() as top:
            const = top.enter_context(tc.tile_pool(name="const", bufs=1))
            ln = top.enter_context(tc.tile_pool(name="ln", bufs=_env("KB_LN", 2)))
            lns = top.enter_context(tc.tile_pool(name="lns", bufs=_env("KB_LNS", 4)))
            ps = top.enter_context(tc.tile_pool(name="ps", bufs=_env("KB_PS", 6), space="PSUM"))
            tps = top.enter_context(tc.tile_pool(name="tps", bufs=_env("KB_TPS", 2), space="PSUM"))
            dram = top.enter_context(tc.tile_pool(name="dram", bufs=1, space="DRAM"))

            contrib = dram.tile([T, E], F32)
            x_stage = dram.tile([T, E], F32)
            rs_out0 = dram.tile([256, E], F32)
            rs_out1 = dram.tile([256, E], F32)
            rs_outs = [rs_out0, rs_out1]

            ident_bf = const.tile([128, 128], BF16)
            make_identity(nc, ident_bf[:])
            ident_f32 = const.tile([128, 128], F32)
            make_identity(nc, ident_f32[:])
            cmask = const.tile([128, 128], F32)
            make_causal_mask(nc, cmask[:], mask_val=MASK_VAL)
            eps_t = const.tile([128, 1], F32)
            nc.vector.memset(eps_t[:], EPS)
            g1b = const.tile([128, E], F32)
            nc.sync.dma_start(out=g1b[:], in_=bcast(g1[:]))
            b1b = const.tile([128, E], F32)
            nc.sync.dma_start(out=b1b[:], in_=bcast(b1[:]))
            xw_sb = const.tile([128, 1], F32)
            nc.sync.dma_start(out=xw_sb[:], in_=bcast(xw[:]))
            bq_sb = const.tile([128, ET, NH], F32)
            for _h in range(NH):
                nc.sync.dma_start(out=bq_sb[:, :, _h], in_=bq[_h].rearrange(
                    "(ft p) -> p ft", p=128))
            bk_sb = const.tile([128, ET, NH], F32)
            for _h in range(NH):
                nc.sync.dma_start(out=bk_sb[:, :, _h], in_=bk[_h].rearrange(
                    "(ft p) -> p ft", p=128))
            bv_sb = const.tile([128, ET, NH], F32)
            for _h in range(NH):
                nc.sync.dma_start(out=bv_sb[:, :, _h], in_=bv[_h].rearrange(
                    "(ft p) -> p ft", p=128))

            with ExitStack() as attn_phase:
                xmain = attn_phase.enter_context(
                    tc.tile_pool(name="xmain", bufs=1))
                wts = attn_phase.enter_context(tc.tile_pool(name="wts", bufs=1))
                qkv = attn_phase.enter_context(tc.tile_pool(name="qkv", bufs=1))
                abuf = attn_phase.enter_context(tc.tile_pool(name="abuf", bufs=_env("KB_ABUF", 2)))
                attp = attn_phase.enter_context(tc.tile_pool(name="attp", bufs=_env("KB_ATTP", 2)))

                xT0 = xmain.tile([128, ET, 512], BF16)
                xT1 = xmain.tile([128, ET, 512], BF16)
                xTs = [xT0, xT1]
                mha = xmain.tile([128, TT, E], F32)

                # ---- LN1 + transpose to xT; stage xw*x into contrib ----
                for tt in ([] if _abl("ln1") else range(TT)):
                    xt = ln.tile([128, E], F32, tag="xt")
                    nc.sync.dma_start(out=xt[:], in_=x_in[128 * tt : 128 * (tt + 1), :])
                    xn = ln.tile([128, E], F32, tag="xn")
                    _layer_norm_tile(nc, lns, xt, g1b, b1b, xn[:], eps_t, apply_gb)
                    xnm = ln.tile([128, E], F32, tag="xnm")
                    nc.vector.tensor_scalar_mul(xnm[:], xn[:], xw_sb[:, 0:1])
                    nc.sync.dma_start(out=x_stage[128 * tt : 128 * (tt + 1), :],
                                      in_=xnm[:])
                    if _dbg:
                        nc.sync.dma_start(
                            out=dbg_x[128 * tt : 128 * (tt + 1), :], in_=xn[:])
                    for et in range(ET):
                        tp = tps.tile([128, 128], F32, tag="tp")
                        nc.tensor.transpose(tp[:], xn[:, 128 * et : 128 * (et + 1)],
                                            ident_f32[:])
                        nc.vector.tensor_copy(
                            xTs[tt // 4][:, et, 128 * (tt % 4) : 128 * (tt % 4 + 1)],
                            tp[:])

                # ---- per-head QKV + attention + partial c_proj ----
                for h in range(NH):
                    wq_sb = wts.tile([128, ET, E], BF16, tag="wq_sb")
                    if not _abl("wdma"): nc.sync.dma_start(out=wq_sb[:], in_=wq[h].rearrange(
                        "(et p) f -> p et f", p=128))
                    wk_sb = wts.tile([128, ET, E], BF16, tag="wk_sb")
                    if not _abl("wdma"): nc.sync.dma_start(out=wk_sb[:], in_=wk[h].rearrange(
                        "(et p) f -> p et f", p=128))
                    wv_sb = wts.tile([128, ET, E], BF16, tag="wv_sb")
                    if not _abl("wdma"): nc.sync.dma_start(out=wv_sb[:], in_=wv[h].rearrange(
                        "(et p) f -> p et f", p=128))
                    wc_sb = wts.tile([128, ET, E], BF16, tag="wc_sb")
                    if not _abl("wdma"): nc.sync.dma_start(out=wc_sb[:], in_=wc[h].rearrange(
                        "(et p) f -> p et f", p=128))

                    qT = qkv.tile([128, ET, T], BF16, tag="qT")
                    kT = qkv.tile([128, ET, T], BF16, tag="kT")
                    v = qkv.tile([128, TT, E], BF16, tag="v")

                    for w_sb, b_sb, dst in (() if _abl("proj") else ((wq_sb, bq_sb, qT), (wk_sb, bk_sb, kT))):
                        for ft in range(ET):
                            for tc2 in range(T // 512):
                                pt = ps.tile([128, 512], F32, tag="ps")
                                for et in range(ET):
                                    nc.tensor.matmul(
                                        pt[:],
                                        w_sb[:, et, 128 * ft : 128 * (ft + 1)],
                                        xTs[tc2][:, et, :],
                                        start=(et == 0), stop=(et == ET - 1))
                                nc.vector.tensor_scalar_add(
                                    dst[:, ft, 512 * tc2 : 512 * (tc2 + 1)],
                                    pt[:], b_sb[:, ft : ft + 1, h])
                    if _dbg and h == 0:
                        for ft in range(ET):
                            qf = ln.tile([128, T], F32, tag="dbgq")
                            nc.vector.tensor_copy(qf[:], qT[:, ft, :])
                            nc.sync.dma_start(
                                out=dbg_qT[128 * ft : 128 * (ft + 1), :], in_=qf[:])
                    for tt in ([] if _abl("proj") else range(TT)):
                        for eo, w in EO_CHUNKS:
                            pt = ps.tile([128, 512], F32, tag="ps")
                            for et in range(ET):
                                nc.tensor.matmul(
                                    pt[:, :w],
                                    xTs[tt // 4][:, et,
                                                 128 * (tt % 4) : 128 * (tt % 4 + 1)],
                                    wv_sb[:, et, eo : eo + w],
                                    start=(et == 0), stop=(et == ET - 1))
                            nc.vector.tensor_copy(v[:, tt, eo : eo + w], pt[:, :w])

                    # ---- attention: software-pipelined over s-tiles so
                    # scores(si) overlap softmax+transpose(si-1) on PE ----
                    expTs = [None, None]
                    headsTs = [None, None]

                    def issue_scores_exp(si):
                        # scores + per-chunk exp (no max-subtraction: scores
                        # are O(1) pre-scale, exp((s + mask)*SCALE) is safe and
                        # masked entries underflow to exactly 0). Row-sums come
                        # free via the ACT accumulator.
                        width = 128 * (si + 1)
                        nch = (width + 511) // 512
                        att = attp.tile([128, T], BF16, tag="att")
                        ds = []
                        for j in range(nch):
                            wj = min(512, width - 512 * j)
                            pt = ps.tile([128, 512], F32, tag="ps")
                            for ft in range(ET):
                                nc.tensor.matmul(
                                    pt[:, :wj],
                                    qT[:, ft, 128 * si : 128 * (si + 1)],
                                    kT[:, ft, 512 * j : 512 * j + wj],
                                    start=(ft == 0), stop=(ft == ET - 1))
                            if j == nch - 1:
                                off = wj - 128
                                nc.vector.tensor_add(pt[:, off : off + 128],
                                                     pt[:, off : off + 128],
                                                     cmask[:])
                            dj = lns.tile([128, 1], F32, tag=f"sm_d{j}")
                            nc.scalar.activation(
                                out=att[:, 512 * j : 512 * j + wj],
                                in_=pt[:, :wj], func=AF.Exp,
                                scale=SCALE, accum_out=dj[:])
                            ds.append(dj)
                        return att, ds

                    def issue_norm_transpose(si, att, ds):
                        sl = si % 4
                        width = 128 * (si + 1)
                        expT = expTs[si // 4]
                        d = ds[0]
                        if len(ds) > 1:
                            nc.vector.tensor_add(d[:], d[:], ds[1][:])
                        if _dbg and h == 0 and si == 0:
                            af = ln.tile([128, T], F32, tag="dbgq")
                            nc.vector.memset(af[:], 0.0)
                            nc.vector.tensor_copy(af[:, :width], att[:, :width])
                            nc.sync.dma_start(out=dbg_att[:], in_=af[:])
                        recip = lns.tile([128, 1], F32, tag="sm_recip")
                        nc.vector.reciprocal(recip[:], d[:])
                        nc.vector.tensor_scalar_mul(att[:, :width],
                                                    att[:, :width], recip[:])
                        for k in range(si + 1):
                            tp = tps.tile([128, 128], BF16, tag="tp")
                            nc.tensor.transpose(
                                tp[:], att[:, 128 * k : 128 * (k + 1)],
                                ident_bf[:])
                            nc.vector.tensor_copy(
                                expT[:, k, 128 * sl : 128 * (sl + 1)], tp[:])

                    def issue_av_cproj(sc):
                        expT = expTs[sc]
                        headsT = headsTs[sc]
                        K = 4 * (sc + 1)
                        for ft in ([] if _abl("av") else range(ET)):
                            pt = ps.tile([128, 512], F32, tag="ps")
                            for k in range(K):
                                nc.tensor.matmul(
                                    pt[:], v[:, k, 128 * ft : 128 * (ft + 1)],
                                    expT[:, k, :],
                                    start=(k == 0), stop=(k == K - 1))
                            nc.vector.tensor_scalar_add(
                                headsT[:, ft, :], pt[:], bv_sb[:, ft : ft + 1, h])
                        for ss in ([] if _abl("cproj") else range(4)):
                            ttg = 4 * sc + ss
                            for eo, w in EO_CHUNKS:
                                pt = ps.tile([128, 512], F32, tag="ps")
                                for ft in range(ET):
                                    nc.tensor.matmul(
                                        pt[:, :w],
                                        headsT[:, ft, 128 * ss : 128 * (ss + 1)],
                                        wc_sb[:, ft, eo : eo + w],
                                        start=(ft == 0), stop=(ft == ET - 1))
                                if h == 0:
                                    nc.vector.tensor_copy(
                                        mha[:, ttg, eo : eo + w], pt[:, :w])
                                else:
                                    nc.vector.tensor_add(
                                        mha[:, ttg, eo : eo + w],
                                        mha[:, ttg, eo : eo + w], pt[:, :w])

                    prev_sm = None
                    for step in range(9):
                        if _abl("attn"):
                            break
                        if step < 8:
                            if step % 4 == 0:
                                sc = step // 4
                                expTs[sc] = abuf.tile([128, TT, 512], BF16,
                                                      tag="expT", name="expT")
                                nc.vector.memset(expTs[sc][:], 0.0)
                                headsTs[sc] = abuf.tile([128, ET, 512], BF16,
                                                        tag="headsT",
                                                        name="headsT")
                            cur_sm = issue_scores_exp(step)
                        if step > 0 and not _abl("sm"):
                            issue_norm_transpose(step - 1, *prev_sm)
                        if step == 4:
                            issue_av_cproj(0)
                        if step == 8:
                            issue_av_cproj(1)
                        prev_sm = cur_sm
                # ---- contribution = mha + staged xw*x ; single DMA out ----
                for tt in ([] if _abl("contrib") else range(TT)):
                    xr = ln.tile([128, E], F32, tag="xt")
                    nc.sync.dma_start(
                        out=xr[:], in_=x_stage[128 * tt : 128 * (tt + 1), :])
                    nc.vector.tensor_add(mha[:, tt, :], mha[:, tt, :], xr[:])
                if not _abl("contrib"):
                    for half in range(2):
                        nc.sync.dma_start(
                            out=contrib[512 * half : 512 * (half + 1), :]
                            .rearrange("(tt p) e -> p tt e", p=128),
                            in_=mha[:, 4 * half : 4 * (half + 1), :])
                if _dbg:
                    nc.sync.dma_start(out=dbg_contrib[:], in_=contrib[:])

            if not _abl("coll"):
                for half in range(2):
                    nc.gpsimd.collective_compute(
                        "ReduceScatter",
                        mybir.AluOpType.add,
                        replica_groups=GROUPS,
                        ins=[contrib[512 * half : 512 * (half + 1), :].opt()],
                        outs=[rs_outs[half].opt()],
                    )

            # ---- post phase: y = rs + bc, LN2, MLP, out ----
            with ExitStack() as post_phase:
                postc = post_phase.enter_context(
                    tc.tile_pool(name="postc", bufs=1))
                mlpw = post_phase.enter_context(
                    tc.tile_pool(name="mlpw", bufs=1))
                mlpa = post_phase.enter_context(
                    tc.tile_pool(name="mlpa", bufs=2))
                outp = post_phase.enter_context(
                    tc.tile_pool(name="outp", bufs=_env("KB_OUTP", 2)))

                bcb = postc.tile([128, E], F32)
                nc.sync.dma_start(out=bcb[:], in_=bcast(bc[:]))
                bpb = postc.tile([128, E], F32)
                nc.sync.dma_start(out=bpb[:], in_=bcast(bp[:]))
                g2b = postc.tile([128, E], F32)
                nc.sync.dma_start(out=g2b[:], in_=bcast(g2[:]))
                b2b = postc.tile([128, E], F32)
                nc.sync.dma_start(out=b2b[:], in_=bcast(b2[:]))
                bfc_sb = postc.tile([128, FHT], F32)
                nc.sync.dma_start(out=bfc_sb[:], in_=bfc.ap().rearrange(
                    "(ft p) -> p ft", p=128))

                wfc_sb = mlpw.tile([128, ET, FH], BF16)
                nc.sync.dma_start(out=wfc_sb[:], in_=wfc.ap().rearrange(
                    "(et p) f -> p et f", p=128))
                wp_sb = mlpw.tile([128, FHT, E], BF16)
                nc.sync.dma_start(out=wp_sb[:], in_=wp.ap().rearrange(
                    "(ft p) e -> p ft e", p=128))

                for half in range(2):
                    rs_sb = mlpa.tile([128, 2, E], F32, tag="rs_sb")
                    nc.sync.dma_start(out=rs_sb[:], in_=rs_outs[half].rearrange(
                        "(tt p) e -> p tt e", p=128))
                    y2 = mlpa.tile([128, 2, E], F32, tag="y2")
                    y2T = mlpa.tile([128, ET, 256], BF16, tag="y2T")
                    hT = mlpa.tile([128, FHT, 256], BF16, tag="hT")

                    for ss in range(2):
                        yt = ln.tile([128, E], F32, tag="xt")
                        nc.vector.tensor_add(yt[:], rs_sb[:, ss, :], bcb[:])
                        _layer_norm_tile(nc, lns, yt, g2b, b2b, y2[:, ss, :],
                                         eps_t, apply_gb)
                        for et in range(ET):
                            tp = tps.tile([128, 128], F32, tag="tp")
                            nc.tensor.transpose(
                                tp[:], y2[:, ss, 128 * et : 128 * (et + 1)],
                                ident_f32[:])
                            nc.vector.tensor_copy(
                                y2T[:, et, 128 * ss : 128 * (ss + 1)], tp[:])

                    for fht in ([] if _abl("mlp") else range(FHT)):
                        pt = ps.tile([128, 512], F32, tag="ps")
                        for et in range(ET):
                            nc.tensor.matmul(
                                pt[:, :256],
                                wfc_sb[:, et, 128 * fht : 128 * (fht + 1)],
                                y2T[:, et, :], start=(et == 0),
                                stop=(et == ET - 1))
                        nc.scalar.activation(out=hT[:, fht, :], in_=pt[:, :256],
                                             func=AF.Gelu,
                                             bias=bfc_sb[:, fht : fht + 1],
                                             scale=1.0)

                    for ss in ([] if _abl("mlp") else range(2)):
                        o_t = outp.tile([128, E], F32, tag="o_t")
                        for eo, w in EO_CHUNKS:
                            pt = ps.tile([128, 512], F32, tag="ps")
                            for fht in range(FHT):
                                nc.tensor.matmul(
                                    pt[:, :w],
                                    hT[:, fht, 128 * ss : 128 * (ss + 1)],
                                    wp_sb[:, fht, eo : eo + w],
                                    start=(fht == 0), stop=(fht == FHT - 1))
                            nc.vector.tensor_add(o_t[:, eo : eo + w], pt[:, :w],
                                                 y2[:, ss, eo : eo + w])
                        nc.vector.tensor_add(o_t[:], o_t[:], bpb[:])
                        nc.sync.dma_start(
                            out=out[256 * half + 128 * ss :
                                    256 * half + 128 * (ss + 1), :],
                            in_=o_t[:])

    nc.compile()
    return nc


def _get_nc():
    global _NC
    if _NC is None:
        _NC = _build()
    return _NC


def kernel(**inputs):
    inp = {k: np.asarray(v) for k, v in inputs.items()}
    nc = _get_nc()

    def b(x):
        return np.ascontiguousarray(x).astype(ml_dtypes.bfloat16)

    def f(x):
        return np.ascontiguousarray(x, dtype=np.float32)

    Wc_h = inp["Wc"].reshape(H, E, E)
    in_maps = []
    for c in range(N_CORES):
        hg, bg = c // 4, c % 4
        hs = slice(NH * hg, NH * (hg + 1))
        in_maps.append({
            "x_in": f(inp["inputs"][bg]),
            "g1": f(inp["g1"]), "b1": f(inp["b1"]),
            "g2": f(inp["g2"]), "b2": f(inp["b2"]),
            "wq": b(inp["Wq"][hs]), "wk": b(inp["Wk"][hs]),
            "wv": b(inp["Wv"][hs]), "wc": b(Wc_h[hs]),
            "bq": f(inp["bq"][hs]), "bk": f(inp["bk"][hs]),
            "bv": f(inp["bv"][hs]),
            "bc": f(inp["bc"]),
            "xw": np.array([1.0 if hg == 0 else 0.0], np.float32),
            "wfc": b(inp["Wfc"]), "bfc": f(inp["bfc"]),
            "wp": b(inp["Wp"]), "bp": f(inp["bp"]),
        })

    res = run_bass_kernel_spmd(nc, in_maps, list(range(N_CORES)))
    out = np.zeros((N, S, E), np.float32)
    for c in range(N_CORES):
        hg, bg = c // 4, c % 4
        o = res.results[c]["out"]
        out[bg, 256 * hg : 256 * (hg + 1)] = o[0:256]
        out[bg, 512 + 256 * hg : 512 + 256 * (hg + 1)] = o[256:512]
    return out
